# revision 6
# baseline (speedup 1.0000x reference)
"""Bi-attention kernel for Trainium2 (8 NeuronCores, data-parallel over batch).

Per-core computation (B=1 slice, Lc=512, Lq=64, D=256):
  score[i,j] = c_i.w_c + q_j.w_q + sum_d c[i,d] q[j,d] w_p[d] + b - 1e30*(1-mask[j])
  h = softmax_j(score);  U[i] = sum_j h[i,j] * (q_j.w_mem)
  u = softmax_i(max_j score);  H = sum_i u[i] * (c_i.w_in)
  G[i] = [ctx1[i], U[i], ctx1[i]*U[i], U[i]*H]

Mapping:
  - context is PE-transposed (identity matmul) to get the contraction dim (D)
    onto partitions; the score matmul then computes, per 128-row chunk,
    [128, 66] = scores (64 cols) | c.w_c | c.w_in, with the per-column
    constants (q_j.w_q + b + mask term) added via a K=1 ones-row matmul.
  - row softmax: DVE reduce_max(negate) -> ACT Exp(bias=-max, accum_out=den)
    -> DVE fused multiply-reduce against broadcast q1 -> reciprocal/mul.
  - softmax over i (partition dim): exp of per-chunk maxes, partition-sums
    via ones-column matmul, tiny scalar division, broadcast back via
    ones-row matmul.
"""

import sys

for _p in ("/opt/trn_rl_repo", "/root/.axon_site/_ro/trn_rl_repo"):
    if _p not in sys.path:
        sys.path.append(_p)

import numpy as np

import concourse.bacc as bacc
import concourse.tile as tile
from concourse import mybir
from concourse.bass_utils import run_bass_kernel_spmd

B, LC, LQ, D = 8, 512, 64, 256
NEG_BIG = 1e30
NCHUNK = LC // 128  # 4 chunks of 128 context rows
KD = D // 128  # 2 contraction chunks
F32 = mybir.dt.float32
I32 = mybir.dt.int32
AF = mybir.ActivationFunctionType
ALU = mybir.AluOpType
AX = mybir.AxisListType


def build_nc():
    nc = bacc.Bacc("TRN2", target_bir_lowering=False, debug=False)

    ctx_d = nc.dram_tensor("context", [LC, D], F32, kind="ExternalInput")
    q_d = nc.dram_tensor("question", [LQ, D], F32, kind="ExternalInput")
    mask_d = nc.dram_tensor("mask", [1, LQ], I32, kind="ExternalInput")
    attw_d = nc.dram_tensor("att_w", [3 * D], F32, kind="ExternalInput")
    attb_d = nc.dram_tensor("att_b", [1, 1], F32, kind="ExternalInput")
    win_d = nc.dram_tensor("w_in", [D], F32, kind="ExternalInput")
    wmem_d = nc.dram_tensor("w_mem", [D], F32, kind="ExternalInput")
    ident_d = nc.dram_tensor("ident", [128, 128], F32, kind="ExternalInput")
    g_d = nc.dram_tensor("G", [LC, 4], F32, kind="ExternalOutput")

    with tile.TileContext(nc) as tc:
        with (
            tc.tile_pool(name="singles", bufs=1) as singles,
            tc.tile_pool(name="ctxp", bufs=4) as ctxp,
            tc.tile_pool(name="cts", bufs=1) as cts,
            tc.tile_pool(name="work", bufs=2) as work,
            tc.tile_pool(name="gpool", bufs=NCHUNK) as gpool,
            tc.tile_pool(name="ps_tr", bufs=2, space="PSUM") as ps_tr,
            tc.tile_pool(name="ps_sc", bufs=2, space="PSUM") as ps_sc,
            tc.tile_pool(name="ps_misc", bufs=1, space="PSUM") as ps_misc,
        ):
            # ---- constants / params ----
            ident = singles.tile([128, 128], F32)
            nc.sync.dma_start(out=ident, in_=ident_d[:, :])
            ones_row = singles.tile([1, 128], F32)
            nc.gpsimd.memset(ones_row, 1.0)
            ones_col = singles.tile([128, 1], F32)
            nc.gpsimd.memset(ones_col, 1.0)
            attb = singles.tile([1, 1], F32)
            nc.sync.dma_start(out=attb, in_=attb_d[:, :])
            mask_i = singles.tile([1, LQ], I32)
            nc.sync.dma_start(out=mask_i, in_=mask_d[:, :])

            # per d-chunk params: w_p column, [w_q | w_mem] pair
            wp = []
            wqm = []
            for k in range(KD):
                wp_k = singles.tile([128, 1], F32, tag=f"wp{k}")
                nc.sync.dma_start(
                    out=wp_k, in_=attw_d[2 * D + 128 * k : 2 * D + 128 * (k + 1)]
                )
                wp.append(wp_k)
                wq_k = singles.tile([128, 1], F32, tag=f"wq{k}")
                nc.sync.dma_start(
                    out=wq_k, in_=attw_d[D + 128 * k : D + 128 * (k + 1)]
                )
                wmem_k = singles.tile([128, 1], F32, tag=f"wmem{k}")
                nc.sync.dma_start(
                    out=wmem_k, in_=wmem_d[128 * k : 128 * (k + 1)]
                )
                wqm.append((wq_k, wmem_k))

            # ---- question transpose: qT_k [128(d), 64(j)] ----
            qsb = singles.tile([LQ, D], F32)
            nc.sync.dma_start(out=qsb, in_=q_d[:, :])
            qt = []
            for k in range(KD):
                qt_ps = ps_misc.tile([128, LQ], F32, tag="early")
                nc.tensor.transpose(
                    qt_ps, qsb[:, 128 * k : 128 * (k + 1)], ident[:LQ, :LQ]
                )
                qt_k = singles.tile([128, LQ], F32, tag=f"qt{k}")
                nc.vector.tensor_copy(qt_k, qt_ps)
                qt.append(qt_k)

            # rhsA_k [128, 66]: cols 0:64 = w_p * qT, col 64 = w_c, col 65 = w_in
            rhsA = []
            for k in range(KD):
                rhsA_k = singles.tile([128, LQ + 2], F32, tag=f"rhsA{k}")
                nc.sync.dma_start(
                    out=rhsA_k[:, LQ : LQ + 1],
                    in_=attw_d[128 * k : 128 * (k + 1)],
                )
                nc.sync.dma_start(
                    out=rhsA_k[:, LQ + 1 : LQ + 2],
                    in_=win_d[128 * k : 128 * (k + 1)],
                )
                nc.vector.tensor_scalar_mul(rhsA_k[:, 0:LQ], qt[k], wp[k])
                rhsA.append(rhsA_k)

            # ---- sq/q1 rows: [1, 64] each = w.T @ qT ----
            sq_ps = ps_misc.tile([1, LQ], F32, tag="early", name="sq_ps")
            for k in range(KD):
                nc.tensor.matmul(
                    sq_ps, wqm[k][0], qt[k], start=(k == 0), stop=(k == KD - 1)
                )
            q1_ps = ps_misc.tile([1, LQ], F32, tag="earlyb", name="q1_ps")
            for k in range(KD):
                nc.tensor.matmul(
                    q1_ps, wqm[k][1], qt[k], start=(k == 0), stop=(k == KD - 1)
                )

            # row_vec [1, 66]: cols j = sq[j] + b - 1e30*(1-mask[j]); 64,65 = 0
            row_vec = singles.tile([1, LQ + 2], F32)
            nc.gpsimd.memset(row_vec, 0.0)
            maskf = singles.tile([1, LQ], F32)
            nc.vector.tensor_copy(maskf, mask_i)
            maskt = singles.tile([1, LQ], F32)
            nc.vector.tensor_scalar(
                maskt, maskf, NEG_BIG, -NEG_BIG, op0=ALU.mult, op1=ALU.add
            )
            sqb = singles.tile([1, LQ], F32)
            nc.vector.tensor_scalar_add(sqb, sq_ps, attb[0:1, 0:1])
            nc.vector.tensor_add(row_vec[0:1, 0:LQ], maskt, sqb)

            # q1 broadcast to all partitions: [128, 64] in PSUM
            q1row = singles.tile([1, LQ], F32)
            nc.vector.tensor_copy(q1row, q1_ps)
            q1bc_ps = ps_misc.tile([128, LQ], F32, tag="q1bc")
            nc.tensor.matmul(q1bc_ps, ones_row, q1row, start=True, stop=True)

            # ---- context load + transpose: cT_k [128(d), 512(i)] ----
            ctx_tiles = []
            for c in range(NCHUNK):
                ct = ctxp.tile([128, D], F32, tag="ctx")
                nc.sync.dma_start(out=ct, in_=ctx_d[128 * c : 128 * (c + 1), :])
                ctx_tiles.append(ct)
            cT = [cts.tile([128, LC], F32, tag=f"cT{k}", name=f"cT{k}") for k in range(KD)]
            for c in range(NCHUNK):
                for k in range(KD):
                    tr_ps = ps_tr.tile([128, 128], F32, tag="tr")
                    nc.tensor.transpose(
                        tr_ps,
                        ctx_tiles[c][:, 128 * k : 128 * (k + 1)],
                        ident,
                    )
                    nc.vector.tensor_copy(cT[k][:, 128 * c : 128 * (c + 1)], tr_ps)

            # ---- per-chunk: score matmul + row softmax + U ----
            m_all = singles.tile([128, NCHUNK], F32)
            ctx1_all = singles.tile([128, NCHUNK], F32)
            g_tiles = []
            for c in range(NCHUNK):
                sc_ps = ps_sc.tile([128, LQ + 2], F32, tag="score")
                for k in range(KD):
                    nc.tensor.matmul(
                        sc_ps,
                        cT[k][:, 128 * c : 128 * (c + 1)],
                        rhsA[k],
                        start=(k == 0),
                        stop=False,
                    )
                nc.tensor.matmul(sc_ps, ones_row, row_vec, start=False, stop=True)

                t_ap = sc_ps[:, 0:LQ]
                nrmax = work.tile([128, 1], F32, tag="nrmax")
                nc.vector.tensor_reduce(nrmax, t_ap, AX.X, ALU.max, negate=True)
                # m = sc + rowmax = sc - nrmax
                nc.vector.tensor_sub(
                    m_all[:, c : c + 1], sc_ps[:, LQ : LQ + 1], nrmax
                )
                e_t = work.tile([128, LQ], F32, tag="e")
                den = work.tile([128, 1], F32, tag="den")
                nc.scalar.activation(
                    e_t, t_ap, AF.Exp, bias=nrmax, scale=1.0, accum_out=den
                )
                num = work.tile([128, 1], F32, tag="num")
                scratch = work.tile([128, LQ], F32, tag="scratch")
                nc.vector.tensor_mul(scratch, e_t, q1bc_ps)
                nc.vector.reduce_sum(num, scratch, axis=AX.X, op=ALU.add)
                rden = work.tile([128, 1], F32, tag="rden")
                nc.vector.reciprocal(rden, den)
                g_t = gpool.tile([128, 4], F32, tag="g")
                nc.vector.tensor_mul(g_t[:, 1:2], num, rden)  # U
                nc.scalar.copy(g_t[:, 0:1], sc_ps[:, LQ + 1 : LQ + 2])  # ctx1
                nc.scalar.copy(ctx1_all[:, c : c + 1], sc_ps[:, LQ + 1 : LQ + 2])
                nc.vector.tensor_mul(
                    g_t[:, 2:3], sc_ps[:, LQ + 1 : LQ + 2], g_t[:, 1:2]
                )  # ctx1*U
                g_tiles.append(g_t)

            # ---- u_aware softmax over i (512 values) + H ----
            exu = singles.tile([128, 2 * NCHUNK], F32)
            nc.scalar.activation(exu[:, 0:NCHUNK], m_all, AF.Exp)
            nc.vector.tensor_mul(exu[:, NCHUNK : 2 * NCHUNK], exu[:, 0:NCHUNK],
                                 ctx1_all)
            hsum_ps = ps_misc.tile([1, 2 * NCHUNK], F32, tag="late")
            nc.tensor.matmul(hsum_ps, ones_col, exu, start=True, stop=True)
            den_u = singles.tile([1, 1], F32)
            num_u = singles.tile([1, 1], F32)
            nc.vector.reduce_sum(den_u, hsum_ps[0:1, 0:NCHUNK], axis=AX.X,
                                 op=ALU.add)
            nc.vector.reduce_sum(num_u, hsum_ps[0:1, NCHUNK : 2 * NCHUNK],
                                 axis=AX.X, op=ALU.add)
            rden_u = singles.tile([1, 1], F32)
            nc.vector.reciprocal(rden_u, den_u)
            h_sb = singles.tile([1, 1], F32)
            nc.vector.tensor_mul(h_sb, num_u, rden_u)
            hbc_ps = ps_misc.tile([128, 1], F32, tag="late", name="hbc_ps")
            nc.tensor.matmul(hbc_ps, ones_row, h_sb, start=True, stop=True)

            for c in range(NCHUNK):
                g_t = g_tiles[c]
                nc.vector.tensor_mul(g_t[:, 3:4], g_t[:, 1:2], hbc_ps)  # U*H
                nc.sync.dma_start(
                    out=g_d[128 * c : 128 * (c + 1), :], in_=g_t
                )

    nc.finalize()
    return nc


_NC = None


def _get_nc():
    global _NC
    if _NC is None:
        _NC = build_nc()
    return _NC


def make_in_maps(context, question, mask, att_w, att_b, w_in, w_mem):
    context = np.asarray(context, np.float32)
    question = np.asarray(question, np.float32)
    mask = np.asarray(mask, np.int32)
    att_w = np.asarray(att_w, np.float32)
    att_b = np.asarray(att_b, np.float32)
    w_in = np.asarray(w_in, np.float32)
    w_mem = np.asarray(w_mem, np.float32)
    ident = np.eye(128, dtype=np.float32)
    return [
        {
            "context": context[b],
            "question": question[b],
            "mask": mask[b][None, :],
            "att_w": att_w,
            "att_b": att_b.reshape(1, 1),
            "w_in": w_in,
            "w_mem": w_mem,
            "ident": ident,
        }
        for b in range(B)
    ]


def kernel(context, question, mask, att_w, att_b, w_in, w_mem):
    nc = _get_nc()
    in_maps = make_in_maps(context, question, mask, att_w, att_b, w_in, w_mem)
    res = run_bass_kernel_spmd(nc, in_maps, core_ids=list(range(B)))
    return np.stack([res.results[c]["G"] for c in range(B)], axis=0)
